# revision 5
# baseline (speedup 1.0000x reference)
"""BiRNN language model on 8 Trainium2 NeuronCores (v3).

Model (see reference): emb lookup -> two tiny 16-wide RNNs (L->R and R->L,
collecting pre-update states) -> logits = [hLR|hRL] @ W_ho.T + b_ho over a
50257 vocab -> log_softmax.  Output [64, 32, 50257] (~412 MB) dominates:
memory-bound regime.

Sharding: data-parallel over batch (B=32 -> 4 columns/core).  Per core:
  1. W_aug = [W_ho.T; b_ho] is packed [97, VPAD/2] bf16 (even vocab chunks
     on partitions 0:33, odd on 64:97) so its one-time DMA uses 97 DMA
     partition lanes instead of 33, and stays resident in SBUF (~51 KB/part).
     haug is replicated to partitions 0:33 and 64:97 to match (matmul lhsT
     and rhs must share a partition base).
  2. Embeddings gathered twice (forward + step-reversed); each RNN direction
     runs as one K=48 matmul + one tanh per step: lhsT = [W_h.T; 0; I] reads
     [h; 0; xproj] so the xproj add costs nothing (no psum prefill).  The
     two chains interleave so engines stay busy.  Pre-update states are
     collected; RL states are copied into word order on the fly.
  3. pass1 per 128-row group rc: 4 chunk matmuls into a [128,2048] psum
     tile -> ACT exp with accum_out (per-row partial sums) -> DVE copies
     the raw logits to a bf16 SBUF cache ring (25 slots).  PE runs at its
     observed 1.2 GHz streaming rate; no recompute pass.
  4. pass2 per rc: out = cache + (-ln(sum)) as 4x-mode DVE tensor_scalar
     adds into bf16 staging tiles, DMA'd to HBM.  Phases pipeline:
     A = pass1(rc0); B = pass1(rc1) || pass2(rc0)+store; C = pass2(rc1).
  5. Output is bf16 in HBM (halves store traffic; ~2e-3 rel error, well
     inside tolerance); the host upcasts to f32.
No collectives; the host concatenates the 8 batch slices.
"""

import sys

sys.path.insert(0, "/opt/trn_rl_repo")

from contextlib import ExitStack

import numpy as np

import concourse.bass as bass
import concourse.bacc as bacc
import concourse.tile as tile
from concourse import mybir
from concourse.bass_utils import run_bass_kernel_spmd
from concourse.masks import make_identity

S, B, V, HID, EMB = 64, 32, 50257, 16, 32
NCORES = 8
BL = B // NCORES          # batch columns per core
R = S * BL                # logit rows per core (256 = 2 row-groups of 128)
XA = EMB + 1              # 33: [x; 1] contraction for the xproj precompute
KA = 2 * HID + 1          # 33: [hLR; hRL; 1] contraction for logits
CHUNK = 512               # vocab columns per matmul (one PSUM bank)
NCHUNKS = 100             # pad V to 100 chunks so the W pack is uniform
VPAD = NCHUNKS * CHUNK    # 51200
WHALF = VPAD // 2         # 25600 columns per W partition-group
TILEW = 4 * CHUNK         # psum/cache tile width (4 chunks)
NT = NCHUNKS // 4         # 25 tiles per row-group
GRPT = 4                  # cache tiles per staging/store DMA group (8192 cols)
NG = (NT + GRPT - 1) // GRPT
HC = 3 * HID              # 48: [h; 0; xp] recurrence contraction

f32 = mybir.dt.float32
bf16 = mybir.dt.bfloat16
i32 = mybir.dt.int32
AF = mybir.ActivationFunctionType


def build_nc():
    nc = bacc.Bacc()

    ind_f = nc.declare_dram_parameter("ind_f", [R, 1], i32, isOutput=False)
    ind_r = nc.declare_dram_parameter("ind_r", [R, 1], i32, isOutput=False)
    emb_tab = nc.declare_dram_parameter("emb_tab", [V, EMB], f32, isOutput=False)
    # [W_x.T; b] per direction for the xproj precompute
    wlrx = nc.declare_dram_parameter("wlrx", [XA, HID], f32, isOutput=False)
    wrlx = nc.declare_dram_parameter("wrlx", [XA, HID], f32, isOutput=False)
    # [W_h.T; 0; I] per direction for the recurrence
    wlr48 = nc.declare_dram_parameter("wlr48", [HC, HID], f32, isOutput=False)
    wrl48 = nc.declare_dram_parameter("wrl48", [HC, HID], f32, isOutput=False)
    h0c = nc.declare_dram_parameter("h0c", [HID, BL], f32, isOutput=False)
    # packed W_aug: rows 0:33 even chunks, rows 64:97 odd chunks
    waug = nc.declare_dram_parameter("waug", [97, WHALF], bf16, isOutput=False)
    out = nc.declare_dram_parameter("out", [R, V], bf16, isOutput=True)

    with ExitStack() as ctx:
        tc = ctx.enter_context(tile.TileContext(nc))
        consts = ctx.enter_context(tc.tile_pool(name="consts", bufs=1))
        cpool = ctx.enter_context(tc.tile_pool(name="cpool", bufs=26))
        epool = ctx.enter_context(tc.tile_pool(name="epool", bufs=2))
        opool = ctx.enter_context(tc.tile_pool(name="opool", bufs=2))
        p1 = ctx.enter_context(tc.tile_pool(name="p1", bufs=2, space="PSUM"))

        # ---- small input DMAs first so nothing queues behind the W load ----
        idx = []
        for nm, src in (("if0", ind_f), ("if1", ind_f), ("ir0", ind_r),
                        ("ir1", ind_r)):
            t = consts.tile([128, 1], i32, tag=f"idx_{nm}", name=f"idx_{nm}")
            half = 1 if nm.endswith("1") else 0
            nc.sync.dma_start(out=t[:, :], in_=src[half * 128:(half + 1) * 128, :])
            idx.append(t)
        wlrx_s = consts.tile([XA, HID], f32, tag="wlrx")
        wrlx_s = consts.tile([XA, HID], f32, tag="wrlx")
        wlr48_s = consts.tile([HC, HID], f32, tag="wlr48")
        wrl48_s = consts.tile([HC, HID], f32, tag="wrl48")
        for dst, src in ((wlrx_s, wlrx), (wrlx_s, wrlx),
                         (wlr48_s, wlr48), (wrl48_s, wrl48)):
            nc.sync.dma_start(out=dst[:, :], in_=src[:, :])

        # ---- W_aug resident load (overlaps gather/xproj/RNN) ----
        waug_sb = consts.tile([97, WHALF], bf16, tag="waug")
        nc.sync.dma_start(out=waug_sb[:, :], in_=waug[:, :])

        # ---- gather embeddings, forward and step-reversed ----
        ers = []
        for k, nm in enumerate(("f0", "f1", "r0", "r1")):
            er = consts.tile([128, EMB], f32, tag=f"er_{nm}", name=f"er_{nm}")
            nc.gpsimd.indirect_dma_start(
                out=er[:, :], out_offset=None, in_=emb_tab[:, :],
                in_offset=bass.IndirectOffsetOnAxis(ap=idx[k][:, :1], axis=0))
            ers.append(er)

        ident = consts.tile([128, 128], f32, tag="ident")
        make_identity(nc, ident)

        # ---- x-major layouts with ones row: xa_f, xa_r [33, 256] ----
        xa_f = consts.tile([XA, R], f32, tag="xa_f")
        xa_r = consts.tile([XA, R], f32, tag="xa_r")
        for xa, e0, e1 in ((xa_f, ers[0], ers[1]), (xa_r, ers[2], ers[3])):
            nc.vector.memset(xa[EMB:XA, :], 1.0)
            for half, er in ((0, e0), (1, e1)):
                pt = p1.tile([EMB, 128], f32, tag="p1", name=f"pt{half}")
                nc.tensor.transpose(pt[:, :], er[:, :], ident[:, :])
                nc.vector.tensor_copy(out=xa[0:EMB, half * 128:(half + 1) * 128],
                                      in_=pt[:, :])

        # ---- chain buffers: rows 0:16 = h states, 16:32 = 0, 32:48 = xp ----
        hx_lr = consts.tile([HC, (S + 1) * BL], f32, tag="hx_lr")
        hx_rl = consts.tile([HC, (S + 1) * BL], f32, tag="hx_rl")
        for hx, w_s, xa in ((hx_lr, wlrx_s, xa_f), (hx_rl, wrlx_s, xa_r)):
            nc.vector.memset(hx[:, :], 0.0)
            pp = p1.tile([HID, R], f32, tag="p1", name="xpj")
            nc.tensor.matmul(pp[:, :], lhsT=w_s[:, :], rhs=xa[:, :],
                             start=True, stop=True)
            nc.vector.tensor_copy(out=hx[2 * HID:HC, 0:R], in_=pp[:, :])
        nc.sync.dma_start(out=hx_lr[0:HID, 0:BL], in_=h0c[:, :])
        nc.sync.dma_start(out=hx_rl[0:HID, 0:BL], in_=h0c[:, :])

        # hRL pre-states rearranged into word order as they are produced
        h_rl = consts.tile([HID, R], f32, tag="h_rl")
        nc.vector.tensor_copy(out=h_rl[:, (S - 1) * BL:S * BL],
                              in_=hx_rl[0:HID, 0:BL])

        # ---- the two recurrences, interleaved (63 steps each) ----
        for t in range(S - 1):
            for hx, w48 in ((hx_lr, wlr48_s), (hx_rl, wrl48_s)):
                ps = p1.tile([HID, BL], f32, tag="p1",
                             name=f"r{'l' if hx is hx_lr else 'r'}{t}")
                nc.tensor.matmul(ps[:, :], lhsT=w48[:, :],
                                 rhs=hx[:, t * BL:(t + 1) * BL],
                                 start=True, stop=True)
                nc.scalar.activation(out=hx[0:HID, (t + 1) * BL:(t + 2) * BL],
                                     in_=ps[:, :], func=AF.Tanh)
            j = t + 1
            nc.vector.tensor_copy(
                out=h_rl[:, (S - 1 - j) * BL:(S - j) * BL],
                in_=hx_rl[0:HID, j * BL:(j + 1) * BL])

        # ---- haug = [hLR; hRL; 1] bf16, replicated to partitions 64:97 ----
        haug_f = consts.tile([KA, R], f32, tag="haug_f")
        nc.vector.tensor_copy(out=haug_f[0:HID, :], in_=hx_lr[0:HID, 0:R])
        # rows 16:32 aren't a legal compute-engine write target; DMA can
        nc.sync.dma_start(out=haug_f[HID:2 * HID, :], in_=h_rl[:, :])
        nc.vector.memset(haug_f[2 * HID:KA, :], 1.0)
        haug2 = consts.tile([97, R], bf16, tag="haug2")
        nc.vector.tensor_copy(out=haug2[0:KA, :], in_=haug_f[:, :])
        nc.vector.tensor_copy(out=haug2[64:64 + KA, :], in_=haug_f[:, :])

        # ---- per-row-group softmax state ----
        sums = [consts.tile([128, NT], f32, tag=f"sums{rc}", name=f"sums{rc}")
                for rc in range(2)]
        negl = [consts.tile([128, 1], f32, tag=f"negl{rc}", name=f"negl{rc}")
                for rc in range(2)]

        cache = {}

        def pass1_tile(rc, k, pfx):
            # tile k covers vocab chunks 4k..4k+3; even chunks live on W
            # partitions 0:33, odd on 64:97 (haug replicated to match)
            ps = p1.tile([128, TILEW], f32, tag="p1", name=f"{pfx}{k}")
            for q in range(4):
                base = 0 if q % 2 == 0 else 64
                j = 2 * k + q // 2
                nc.tensor.matmul(
                    ps[:, q * CHUNK:(q + 1) * CHUNK],
                    lhsT=haug2[base:base + KA, rc * 128:(rc + 1) * 128],
                    rhs=waug_sb[base:base + KA, j * CHUNK:(j + 1) * CHUNK],
                    start=True, stop=True)
            ex = epool.tile([128, TILEW], bf16, tag="ex", name=f"ex{pfx}{k}")
            nc.scalar.activation(out=ex[:, :], in_=ps[:, :], func=AF.Exp,
                                 accum_out=sums[rc][:, k:k + 1])
            ca = cpool.tile([128, TILEW], bf16, tag="ca", name=f"ca{pfx}{k}")
            nc.vector.tensor_copy(out=ca[:, :], in_=ps[:, :])
            cache[(rc, k)] = ca

        def finish_negl(rc):
            tot = consts.tile([128, 1], f32, tag=f"tot{rc}", name=f"tot{rc}")
            nc.vector.reduce_sum(out=tot[:, :], in_=sums[rc][:, 0:NT],
                                 axis=mybir.AxisListType.X)
            lnt = consts.tile([128, 1], f32, tag=f"lnt{rc}", name=f"lnt{rc}")
            nc.scalar.activation(out=lnt[:, :], in_=tot[:, :], func=AF.Ln)
            nc.vector.tensor_scalar_mul(out=negl[rc][:, :], in0=lnt[:, :],
                                        scalar1=-1.0)

        st_state = {}

        def pass2_tile(rc, k, on_act):
            ca = cache.pop((rc, k))
            g, kk = divmod(k, GRPT)
            if kk == 0:
                st_state[rc] = opool.tile([128, GRPT * TILEW], bf16, tag="st",
                                          name=f"st{rc}_{g}")
            st = st_state[rc]
            dst = st[:, kk * TILEW:(kk + 1) * TILEW]
            if on_act:
                nc.scalar.activation(out=dst, in_=ca[:, :], func=AF.Identity,
                                     bias=negl[rc][:, 0:1])
            else:
                nc.vector.tensor_scalar_add(out=dst, in0=ca[:, :],
                                            scalar1=negl[rc][:, 0:1])
            if kk == GRPT - 1 or k == NT - 1:
                c0g = g * GRPT * TILEW
                gw = (kk + 1) * TILEW
                cw = min(gw, V - c0g)
                nc.sync.dma_start(
                    out=out[rc * 128:(rc + 1) * 128, c0g:c0g + cw],
                    in_=st[:, 0:cw])

        # ---- phase A: pass1(rc0) ----
        for k in range(NT):
            pass1_tile(0, k, "a")
        finish_negl(0)

        # ---- phase B: pass1(rc1) || pass2(rc0) + store ----
        # pass2(rc0,k) reads cache slot k before pass1(rc1,k)'s copy
        # recycles it, so it must come first in (DVE) program order.
        for k in range(NT):
            pass2_tile(0, k, on_act=(k % 4 == 3))
            pass1_tile(1, k, "b")
        finish_negl(1)

        # ---- phase C: pass2(rc1) ----
        for k in range(NT):
            pass2_tile(1, k, on_act=(k % 3 == 2))

    nc.finalize()
    return nc


_NC = None


def get_nc():
    global _NC
    if _NC is None:
        _NC = build_nc()
    return _NC


def _make_waug(Who, bho):
    # [W_hLR; W_hRL; b_ho] packed two chunks deep: rows 0:33 hold even
    # vocab chunks, rows 64:97 odd chunks.  Pad columns carry bias -1e4 so
    # exp(logit) underflows to exactly 0.
    flat = np.zeros((KA, VPAD), dtype=np.float32)
    flat[0:2 * HID, :V] = Who.T
    flat[2 * HID, :V] = bho
    flat[2 * HID, V:] = -1e4
    packed = np.zeros((97, WHALF), dtype=np.float32)
    c = flat.reshape(KA, NCHUNKS, CHUNK)
    packed[0:KA] = c[:, 0::2, :].reshape(KA, WHALF)
    packed[64:64 + KA] = c[:, 1::2, :].reshape(KA, WHALF)
    return packed


def make_in_maps(**inputs):
    ib = np.asarray(inputs["input_batch"]).astype(np.int32)          # [S, B]
    emb = np.ascontiguousarray(np.asarray(inputs["embedding"], dtype=np.float32))
    Wlr = np.asarray(inputs["W_lr"], dtype=np.float32)               # [16, 48]
    Wrl = np.asarray(inputs["W_rl"], dtype=np.float32)
    blr = np.asarray(inputs["b_lr"], dtype=np.float32).reshape(1, HID)
    brl = np.asarray(inputs["b_rl"], dtype=np.float32).reshape(1, HID)
    Who = np.asarray(inputs["W_ho"], dtype=np.float32)               # [V, 32]
    bho = np.asarray(inputs["b_ho"], dtype=np.float32)               # [V]
    h0 = np.asarray(inputs["h0"], dtype=np.float32)                  # [1, 16]

    def w48(W):
        m = np.zeros((HC, HID), dtype=np.float32)
        m[0:HID] = W[:, EMB:].T
        m[2 * HID:HC] = np.eye(HID, dtype=np.float32)
        return m

    shared = dict(
        emb_tab=emb,
        waug=_make_waug(Who, bho).astype(mybir.dt.np(bf16)),
        wlrx=np.ascontiguousarray(np.concatenate([Wlr[:, :EMB].T, blr], axis=0)),
        wrlx=np.ascontiguousarray(np.concatenate([Wrl[:, :EMB].T, brl], axis=0)),
        wlr48=w48(Wlr),
        wrl48=w48(Wrl),
        h0c=np.ascontiguousarray(np.broadcast_to(h0.T, (HID, BL))),
    )
    in_maps = []
    for c in range(NCORES):
        cols = ib[:, c * BL:(c + 1) * BL]
        ind_f = np.ascontiguousarray(cols.reshape(R, 1))
        ind_r = np.ascontiguousarray(cols[::-1, :].reshape(R, 1))
        in_maps.append({**shared, "ind_f": ind_f, "ind_r": ind_r})
    return in_maps


def assemble(results):
    outs = [np.asarray(results[c]["out"], dtype=np.float32).reshape(S, BL, V)
            for c in range(NCORES)]
    return np.concatenate(outs, axis=1)


def kernel(**inputs):
    in_maps = make_in_maps(**inputs)
    res = run_bass_kernel_spmd(get_nc(), in_maps, list(range(NCORES)))
    return assemble(res.results)


if __name__ == "__main__":
    rng = np.random.default_rng(0)
    stdv = 1.0 / np.sqrt(HID)
    u = lambda *shp: rng.uniform(-stdv, stdv, shp).astype(np.float32)
    demo = dict(
        input_batch=rng.integers(0, V, (S, B)).astype(np.int32),
        embedding=u(V, EMB), W_lr=u(HID, EMB + HID), b_lr=u(HID),
        W_rl=u(HID, EMB + HID), b_rl=u(HID), W_ho=u(V, 2 * HID), b_ho=u(V),
        h0=u(1, HID),
    )
    out_arr = kernel(**demo)
    print(out_arr.shape, out_arr.dtype, float(out_arr[0, 0, :3].sum()))


# revision 10
# speedup vs baseline: 1.7100x; 1.7100x over previous
"""BiRNN language model on 8 Trainium2 NeuronCores (v3).

Model (see reference): emb lookup -> two tiny 16-wide RNNs (L->R and R->L,
collecting pre-update states) -> logits = [hLR|hRL] @ W_ho.T + b_ho over a
50257 vocab -> log_softmax.  Output [64, 32, 50257] (~412 MB) dominates:
memory-bound regime.

Sharding: data-parallel over batch (B=32 -> 4 columns/core).  Per core:
  1. W_aug = [W_ho.T; b_ho] is packed [97, VPAD/2] bf16 (even vocab chunks
     on partitions 0:33, odd on 64:97) so its one-time DMA uses 97 DMA
     partition lanes instead of 33, and stays resident in SBUF (~51 KB/part).
     haug is replicated to partitions 0:33 and 64:97 to match (matmul lhsT
     and rhs must share a partition base).
  2. Embeddings gathered twice (forward + step-reversed); each RNN direction
     runs as one K=48 matmul + one tanh per step: lhsT = [W_h.T; 0; I] reads
     [h; 0; xproj] so the xproj add costs nothing (no psum prefill).  The
     two chains interleave so engines stay busy.  Pre-update states are
     collected; RL states are copied into word order on the fly.
  3. pass1 per 128-row group rc: 4 chunk matmuls into a [128,2048] psum
     tile -> ACT exp with accum_out (per-row partial sums) -> DVE copies
     the raw logits to a bf16 SBUF cache ring (25 slots).  PE runs at its
     observed 1.2 GHz streaming rate; no recompute pass.
  4. pass2 per rc: out = cache + (-ln(sum)) as 4x-mode DVE tensor_scalar
     adds into bf16 staging tiles, DMA'd to HBM.  Phases pipeline:
     A = pass1(rc0); B = pass1(rc1) || pass2(rc0)+store; C = pass2(rc1).
  5. Output is bf16 in HBM (halves store traffic; ~2e-3 rel error, well
     inside tolerance); the host upcasts to f32.
No collectives; the host concatenates the 8 batch slices.
"""

import sys

sys.path.insert(0, "/opt/trn_rl_repo")

from contextlib import ExitStack

import numpy as np

import concourse.bass as bass
import concourse.bacc as bacc
import concourse.tile as tile
from concourse import mybir
from concourse.bass_utils import run_bass_kernel_spmd
from concourse.masks import make_identity

S, B, V, HID, EMB = 64, 32, 50257, 16, 32
NCORES = 8
BL = B // NCORES          # batch columns per core
R = S * BL                # logit rows per core (256 = 2 row-groups of 128)
XA = EMB + 1              # 33: [x; 1] contraction for the xproj precompute
KA = 2 * HID + 1          # 33: [hLR; hRL; 1] contraction for logits
CHUNK = 512               # vocab columns per matmul (one PSUM bank)
NCHUNKS = 100             # pad V to 100 chunks so the W pack is uniform
VPAD = NCHUNKS * CHUNK    # 51200
WHALF = VPAD // 2         # 25600 columns per W partition-group
TILEW = 4 * CHUNK         # psum/cache tile width (4 chunks)
NT = NCHUNKS // 4         # 25 tiles per row-group
GRPT = 4                  # cache tiles per staging/store DMA group (8192 cols)
NG = (NT + GRPT - 1) // GRPT
HC = 3 * HID              # 48: [h; 0; xp] recurrence contraction

f32 = mybir.dt.float32
bf16 = mybir.dt.bfloat16
i32 = mybir.dt.int32
AF = mybir.ActivationFunctionType


def build_nc():
    nc = bacc.Bacc()

    ind_f = nc.declare_dram_parameter("ind_f", [R, 1], i32, isOutput=False)
    ind_r = nc.declare_dram_parameter("ind_r", [R, 1], i32, isOutput=False)
    emb_tab = nc.declare_dram_parameter("emb_tab", [V, EMB], f32, isOutput=False)
    # [W_x.T; b] per direction for the xproj precompute
    wlrx = nc.declare_dram_parameter("wlrx", [XA, HID], f32, isOutput=False)
    wrlx = nc.declare_dram_parameter("wrlx", [XA, HID], f32, isOutput=False)
    # [W_h.T; 0; I] per direction for the recurrence
    wlr48 = nc.declare_dram_parameter("wlr48", [HC, HID], f32, isOutput=False)
    wrl48 = nc.declare_dram_parameter("wrl48", [HC, HID], f32, isOutput=False)
    h0c = nc.declare_dram_parameter("h0c", [HID, BL], f32, isOutput=False)
    # packed W_aug: rows 0:33 even chunks, rows 64:97 odd chunks; padded to
    # 128 partitions because few-partition long-line HBM reads serialize on
    # one SDMA engine (~26 GB/s) while 128-partition transfers go ~348 GB/s
    waug = nc.declare_dram_parameter("waug", [128, WHALF], bf16, isOutput=False)
    out = nc.declare_dram_parameter("out", [R, V], bf16, isOutput=True)

    with ExitStack() as ctx:
        tc = ctx.enter_context(tile.TileContext(nc))
        consts = ctx.enter_context(tc.tile_pool(name="consts", bufs=1))
        cpool = ctx.enter_context(tc.tile_pool(name="cpool", bufs=26))
        epool = ctx.enter_context(tc.tile_pool(name="epool", bufs=2))
        opool = ctx.enter_context(tc.tile_pool(name="opool", bufs=2))
        p1 = ctx.enter_context(tc.tile_pool(name="p1", bufs=2, space="PSUM"))

        # ---- small input DMAs first so nothing queues behind the W load ----
        idx = []
        for nm, src in (("if0", ind_f), ("if1", ind_f), ("ir0", ind_r),
                        ("ir1", ind_r)):
            t = consts.tile([128, 1], i32, tag=f"idx_{nm}", name=f"idx_{nm}")
            half = 1 if nm.endswith("1") else 0
            nc.sync.dma_start(out=t[:, :], in_=src[half * 128:(half + 1) * 128, :])
            idx.append(t)
        wlrx_s = consts.tile([XA, HID], f32, tag="wlrx")
        wrlx_s = consts.tile([XA, HID], f32, tag="wrlx")
        wlr48_s = consts.tile([HC, HID], f32, tag="wlr48")
        wrl48_s = consts.tile([HC, HID], f32, tag="wrl48")
        for dst, src in ((wlrx_s, wlrx), (wrlx_s, wrlx),
                         (wlr48_s, wlr48), (wrl48_s, wrl48)):
            nc.sync.dma_start(out=dst[:, :], in_=src[:, :])

        # ---- chain buffers created early so their h0 DMAs beat the W load
        # into the FIFO sync queue; rows 0:16 = h states, 16:32 = 0 (junk
        # stripe), 32:48 = xproj (filled later) ----
        hx_lr = consts.tile([HC, (S + 1) * BL], f32, tag="hx_lr")
        hx_rl = consts.tile([HC, (S + 1) * BL], f32, tag="hx_rl")
        nc.vector.memset(hx_lr[:, :], 0.0)
        nc.vector.memset(hx_rl[:, :], 0.0)
        nc.sync.dma_start(out=hx_lr[0:HID, 0:BL], in_=h0c[:, :])
        nc.sync.dma_start(out=hx_rl[0:HID, 0:BL], in_=h0c[:, :])

        # ---- W_aug resident load (overlaps gather/xproj/RNN) ----
        waug_sb = consts.tile([128, WHALF], bf16, tag="waug")
        nc.sync.dma_start(out=waug_sb[:, :], in_=waug[:, :])

        # ---- gather embeddings, forward and step-reversed ----
        ers = []
        for k, nm in enumerate(("f0", "f1", "r0", "r1")):
            er = consts.tile([128, EMB], f32, tag=f"er_{nm}", name=f"er_{nm}")
            nc.gpsimd.indirect_dma_start(
                out=er[:, :], out_offset=None, in_=emb_tab[:, :],
                in_offset=bass.IndirectOffsetOnAxis(ap=idx[k][:, :1], axis=0))
            ers.append(er)

        ident = consts.tile([128, 128], f32, tag="ident")
        make_identity(nc, ident)

        # ---- x-major layouts with ones row: xa_f, xa_r [33, 256] ----
        xa_f = consts.tile([XA, R], f32, tag="xa_f")
        xa_r = consts.tile([XA, R], f32, tag="xa_r")
        for xa, e0, e1 in ((xa_f, ers[0], ers[1]), (xa_r, ers[2], ers[3])):
            nc.vector.memset(xa[EMB:XA, :], 1.0)
            for half, er in ((0, e0), (1, e1)):
                pt = p1.tile([EMB, 128], f32, tag="p1", name=f"pt{half}")
                nc.tensor.transpose(pt[:, :], er[:, :], ident[:, :])
                nc.vector.tensor_copy(out=xa[0:EMB, half * 128:(half + 1) * 128],
                                      in_=pt[:, :])

        # ---- xproj into the chain buffers' rows 32:48 ----
        for hx, w_s, xa in ((hx_lr, wlrx_s, xa_f), (hx_rl, wrlx_s, xa_r)):
            pp = p1.tile([HID, R], f32, tag="p1", name="xpj")
            nc.tensor.matmul(pp[:, :], lhsT=w_s[:, :], rhs=xa[:, :],
                             start=True, stop=True)
            nc.vector.tensor_copy(out=hx[2 * HID:HC, 0:R], in_=pp[:, :])

        # hRL pre-states rearranged into word order as they are produced
        h_rl = consts.tile([HID, R], f32, tag="h_rl")
        nc.vector.tensor_copy(out=h_rl[:, (S - 1) * BL:S * BL],
                              in_=hx_rl[0:HID, 0:BL])

        # ---- the two recurrences, interleaved (63 steps each) ----
        for t in range(S - 1):
            for hx, w48 in ((hx_lr, wlr48_s), (hx_rl, wrl48_s)):
                ps = p1.tile([HID, BL], f32, tag="p1",
                             name=f"r{'l' if hx is hx_lr else 'r'}{t}")
                nc.tensor.matmul(ps[:, :], lhsT=w48[:, :],
                                 rhs=hx[:, t * BL:(t + 1) * BL],
                                 start=True, stop=True)
                nc.scalar.activation(out=hx[0:HID, (t + 1) * BL:(t + 2) * BL],
                                     in_=ps[:, :], func=AF.Tanh)
            j = t + 1
            nc.vector.tensor_copy(
                out=h_rl[:, (S - 1 - j) * BL:(S - j) * BL],
                in_=hx_rl[0:HID, j * BL:(j + 1) * BL])

        # ---- haug = [hLR; hRL; 1] bf16, replicated to partitions 64:97 ----
        haug_f = consts.tile([KA, R], f32, tag="haug_f")
        nc.vector.tensor_copy(out=haug_f[0:HID, :], in_=hx_lr[0:HID, 0:R])
        # rows 16:32 aren't a legal compute-engine write target; DMA can
        nc.sync.dma_start(out=haug_f[HID:2 * HID, :], in_=h_rl[:, :])
        nc.vector.memset(haug_f[2 * HID:KA, :], 1.0)
        haug2 = consts.tile([97, R], bf16, tag="haug2")
        nc.vector.tensor_copy(out=haug2[0:KA, :], in_=haug_f[:, :])
        nc.vector.tensor_copy(out=haug2[64:64 + KA, :], in_=haug_f[:, :])

        # ---- per-row-group softmax state ----
        sums = [consts.tile([128, NT], f32, tag=f"sums{rc}", name=f"sums{rc}")
                for rc in range(2)]
        negl = [consts.tile([128, 1], f32, tag=f"negl{rc}", name=f"negl{rc}")
                for rc in range(2)]

        cache = {}

        def pass1_tile(rc, k, pfx):
            # tile k covers vocab chunks 4k..4k+3; even chunks live on W
            # partitions 0:33, odd on 64:97 (haug replicated to match)
            ps = p1.tile([128, TILEW], f32, tag="p1", name=f"{pfx}{k}")
            for q in range(4):
                base = 0 if q % 2 == 0 else 64
                j = 2 * k + q // 2
                nc.tensor.matmul(
                    ps[:, q * CHUNK:(q + 1) * CHUNK],
                    lhsT=haug2[base:base + KA, rc * 128:(rc + 1) * 128],
                    rhs=waug_sb[base:base + KA, j * CHUNK:(j + 1) * CHUNK],
                    start=True, stop=True)
            ex = epool.tile([128, TILEW], bf16, tag="ex", name=f"ex{pfx}{k}")
            nc.scalar.activation(out=ex[:, :], in_=ps[:, :], func=AF.Exp,
                                 accum_out=sums[rc][:, k:k + 1])
            ca = cpool.tile([128, TILEW], bf16, tag="ca", name=f"ca{pfx}{k}")
            nc.vector.tensor_copy(out=ca[:, :], in_=ps[:, :])
            cache[(rc, k)] = ca

        def finish_negl(rc):
            tot = consts.tile([128, 1], f32, tag=f"tot{rc}", name=f"tot{rc}")
            nc.vector.reduce_sum(out=tot[:, :], in_=sums[rc][:, 0:NT],
                                 axis=mybir.AxisListType.X)
            lnt = consts.tile([128, 1], f32, tag=f"lnt{rc}", name=f"lnt{rc}")
            nc.scalar.activation(out=lnt[:, :], in_=tot[:, :], func=AF.Ln)
            nc.vector.tensor_scalar_mul(out=negl[rc][:, :], in0=lnt[:, :],
                                        scalar1=-1.0)

        st_state = {}

        def pass2_tile(rc, k, on_act):
            ca = cache.pop((rc, k))
            g, kk = divmod(k, GRPT)
            if kk == 0:
                st_state[rc] = opool.tile([128, GRPT * TILEW], bf16, tag="st",
                                          name=f"st{rc}_{g}")
            st = st_state[rc]
            dst = st[:, kk * TILEW:(kk + 1) * TILEW]
            if on_act:
                nc.scalar.activation(out=dst, in_=ca[:, :], func=AF.Identity,
                                     bias=negl[rc][:, 0:1])
            else:
                nc.vector.tensor_scalar_add(out=dst, in0=ca[:, :],
                                            scalar1=negl[rc][:, 0:1])
            if kk == GRPT - 1 or k == NT - 1:
                c0g = g * GRPT * TILEW
                gw = (kk + 1) * TILEW
                cw = min(gw, V - c0g)
                nc.sync.dma_start(
                    out=out[rc * 128:(rc + 1) * 128, c0g:c0g + cw],
                    in_=st[:, 0:cw])

        # ---- phase A: pass1(rc0) ----
        for k in range(NT):
            pass1_tile(0, k, "a")
        finish_negl(0)

        # ---- phase B: pass1(rc1) || pass2(rc0) + store ----
        # pass2(rc0,k) reads cache slot k before pass1(rc1,k)'s copy
        # recycles it, so it must come first in (DVE) program order.
        for k in range(NT):
            pass2_tile(0, k, on_act=(k % 4 == 3))
            pass1_tile(1, k, "b")
        finish_negl(1)

        # ---- phase C: pass2(rc1) ----
        for k in range(NT):
            pass2_tile(1, k, on_act=(k % 3 == 2))

    nc.finalize()
    return nc


_NC = None


def get_nc():
    global _NC
    if _NC is None:
        _NC = build_nc()
    return _NC


def _make_waug(Who, bho):
    # [W_hLR; W_hRL; b_ho] packed two chunks deep: rows 0:33 hold even
    # vocab chunks, rows 64:97 odd chunks.  Pad columns carry bias -1e4 so
    # exp(logit) underflows to exactly 0.
    flat = np.zeros((KA, VPAD), dtype=np.float32)
    flat[0:2 * HID, :V] = Who.T
    flat[2 * HID, :V] = bho
    flat[2 * HID, V:] = -1e4
    packed = np.zeros((128, WHALF), dtype=np.float32)
    c = flat.reshape(KA, NCHUNKS, CHUNK)
    packed[0:KA] = c[:, 0::2, :].reshape(KA, WHALF)
    packed[64:64 + KA] = c[:, 1::2, :].reshape(KA, WHALF)
    return packed


def make_in_maps(**inputs):
    ib = np.asarray(inputs["input_batch"]).astype(np.int32)          # [S, B]
    emb = np.ascontiguousarray(np.asarray(inputs["embedding"], dtype=np.float32))
    Wlr = np.asarray(inputs["W_lr"], dtype=np.float32)               # [16, 48]
    Wrl = np.asarray(inputs["W_rl"], dtype=np.float32)
    blr = np.asarray(inputs["b_lr"], dtype=np.float32).reshape(1, HID)
    brl = np.asarray(inputs["b_rl"], dtype=np.float32).reshape(1, HID)
    Who = np.asarray(inputs["W_ho"], dtype=np.float32)               # [V, 32]
    bho = np.asarray(inputs["b_ho"], dtype=np.float32)               # [V]
    h0 = np.asarray(inputs["h0"], dtype=np.float32)                  # [1, 16]

    def w48(W):
        m = np.zeros((HC, HID), dtype=np.float32)
        m[0:HID] = W[:, EMB:].T
        m[2 * HID:HC] = np.eye(HID, dtype=np.float32)
        return m

    shared = dict(
        emb_tab=emb,
        waug=_make_waug(Who, bho).astype(mybir.dt.np(bf16)),
        wlrx=np.ascontiguousarray(np.concatenate([Wlr[:, :EMB].T, blr], axis=0)),
        wrlx=np.ascontiguousarray(np.concatenate([Wrl[:, :EMB].T, brl], axis=0)),
        wlr48=w48(Wlr),
        wrl48=w48(Wrl),
        h0c=np.ascontiguousarray(np.broadcast_to(h0.T, (HID, BL))),
    )
    in_maps = []
    for c in range(NCORES):
        cols = ib[:, c * BL:(c + 1) * BL]
        ind_f = np.ascontiguousarray(cols.reshape(R, 1))
        ind_r = np.ascontiguousarray(cols[::-1, :].reshape(R, 1))
        in_maps.append({**shared, "ind_f": ind_f, "ind_r": ind_r})
    return in_maps


def assemble(results):
    outs = [np.asarray(results[c]["out"], dtype=np.float32).reshape(S, BL, V)
            for c in range(NCORES)]
    return np.concatenate(outs, axis=1)


def kernel(**inputs):
    in_maps = make_in_maps(**inputs)
    res = run_bass_kernel_spmd(get_nc(), in_maps, list(range(NCORES)))
    return assemble(res.results)


if __name__ == "__main__":
    rng = np.random.default_rng(0)
    stdv = 1.0 / np.sqrt(HID)
    u = lambda *shp: rng.uniform(-stdv, stdv, shp).astype(np.float32)
    demo = dict(
        input_batch=rng.integers(0, V, (S, B)).astype(np.int32),
        embedding=u(V, EMB), W_lr=u(HID, EMB + HID), b_lr=u(HID),
        W_rl=u(HID, EMB + HID), b_rl=u(HID), W_ho=u(V, 2 * HID), b_ho=u(V),
        h0=u(1, HID),
    )
    out_arr = kernel(**demo)
    print(out_arr.shape, out_arr.dtype, float(out_arr[0, 0, :3].sum()))


# revision 11
# speedup vs baseline: 1.8007x; 1.0530x over previous
"""BiRNN language model on 8 Trainium2 NeuronCores (v3).

Model (see reference): emb lookup -> two tiny 16-wide RNNs (L->R and R->L,
collecting pre-update states) -> logits = [hLR|hRL] @ W_ho.T + b_ho over a
50257 vocab -> log_softmax.  Output [64, 32, 50257] (~412 MB) dominates:
memory-bound regime.

Sharding: data-parallel over batch (B=32 -> 4 columns/core).  Per core:
  1. W_aug = [W_ho.T; b_ho] is packed [97, VPAD/2] bf16 (even vocab chunks
     on partitions 0:33, odd on 64:97) so its one-time DMA uses 97 DMA
     partition lanes instead of 33, and stays resident in SBUF (~51 KB/part).
     haug is replicated to partitions 0:33 and 64:97 to match (matmul lhsT
     and rhs must share a partition base).
  2. Embeddings gathered twice (forward + step-reversed); each RNN direction
     runs as one K=48 matmul + one tanh per step: lhsT = [W_h.T; 0; I] reads
     [h; 0; xproj] so the xproj add costs nothing (no psum prefill).  The
     two chains interleave so engines stay busy.  Pre-update states are
     collected; RL states are copied into word order on the fly.
  3. pass1 per 128-row group rc: 4 chunk matmuls into a [128,2048] psum
     tile -> ACT exp with accum_out (per-row partial sums) -> DVE copies
     the raw logits to a bf16 SBUF cache ring (25 slots).  PE runs at its
     observed 1.2 GHz streaming rate; no recompute pass.
  4. pass2 per rc: out = cache + (-ln(sum)) as 4x-mode DVE tensor_scalar
     adds into bf16 staging tiles, DMA'd to HBM.  Phases pipeline:
     A = pass1(rc0); B = pass1(rc1) || pass2(rc0)+store; C = pass2(rc1).
  5. Output is bf16 in HBM (halves store traffic; ~2e-3 rel error, well
     inside tolerance); the host upcasts to f32.
No collectives; the host concatenates the 8 batch slices.
"""

import sys

sys.path.insert(0, "/opt/trn_rl_repo")

from contextlib import ExitStack

import numpy as np

import concourse.bass as bass
import concourse.bacc as bacc
import concourse.tile as tile
from concourse import mybir
from concourse.bass_utils import run_bass_kernel_spmd
from concourse.masks import make_identity

S, B, V, HID, EMB = 64, 32, 50257, 16, 32
NCORES = 8
BL = B // NCORES          # batch columns per core
R = S * BL                # logit rows per core (256 = 2 row-groups of 128)
XA = EMB + 1              # 33: [x; 1] contraction for the xproj precompute
KA = 2 * HID + 1          # 33: [hLR; hRL; 1] contraction for logits
CHUNK = 512               # vocab columns per matmul (one PSUM bank)
NCHUNKS = 100             # pad V to 100 chunks so the W pack is uniform
VPAD = NCHUNKS * CHUNK    # 51200
WHALF = VPAD // 2         # 25600 columns per W partition-group
TILEW = 4 * CHUNK         # psum/cache tile width (4 chunks)
NT = NCHUNKS // 4         # 25 tiles per row-group
GRPT = 4                  # cache tiles per staging/store DMA group (8192 cols)
NG = (NT + GRPT - 1) // GRPT
HC = 3 * HID              # 48: [h; 0; xp] recurrence contraction

f32 = mybir.dt.float32
bf16 = mybir.dt.bfloat16
i32 = mybir.dt.int32
AF = mybir.ActivationFunctionType


def build_nc():
    nc = bacc.Bacc()

    ind_f = nc.declare_dram_parameter("ind_f", [R, 1], i32, isOutput=False)
    ind_r = nc.declare_dram_parameter("ind_r", [R, 1], i32, isOutput=False)
    emb_tab = nc.declare_dram_parameter("emb_tab", [V, EMB], f32, isOutput=False)
    # [W_x.T; b] per direction for the xproj precompute
    wlrx = nc.declare_dram_parameter("wlrx", [XA, HID], f32, isOutput=False)
    wrlx = nc.declare_dram_parameter("wrlx", [XA, HID], f32, isOutput=False)
    # [W_h.T; 0; I] per direction for the recurrence
    wlr48 = nc.declare_dram_parameter("wlr48", [HC, HID], f32, isOutput=False)
    wrl48 = nc.declare_dram_parameter("wrl48", [HC, HID], f32, isOutput=False)
    h0c = nc.declare_dram_parameter("h0c", [HID, BL], f32, isOutput=False)
    # packed W_aug: rows 0:33 even chunks, rows 64:97 odd chunks; padded to
    # 128 partitions because few-partition long-line HBM reads serialize on
    # one SDMA engine (~26 GB/s) while 128-partition transfers go ~348 GB/s
    waug = nc.declare_dram_parameter("waug", [128, WHALF], bf16, isOutput=False)
    out = nc.declare_dram_parameter("out", [R, V], bf16, isOutput=True)

    with ExitStack() as ctx:
        tc = ctx.enter_context(tile.TileContext(nc))
        consts = ctx.enter_context(tc.tile_pool(name="consts", bufs=1))
        cpool = ctx.enter_context(tc.tile_pool(name="cpool", bufs=26))
        epool = ctx.enter_context(tc.tile_pool(name="epool", bufs=2))
        opool = ctx.enter_context(tc.tile_pool(name="opool", bufs=2))
        p1 = ctx.enter_context(tc.tile_pool(name="p1", bufs=2, space="PSUM"))

        # ---- small input DMAs first so nothing queues behind the W load ----
        idx = []
        for nm, src in (("if0", ind_f), ("if1", ind_f), ("ir0", ind_r),
                        ("ir1", ind_r)):
            t = consts.tile([128, 1], i32, tag=f"idx_{nm}", name=f"idx_{nm}")
            half = 1 if nm.endswith("1") else 0
            nc.sync.dma_start(out=t[:, :], in_=src[half * 128:(half + 1) * 128, :])
            idx.append(t)
        wlrx_s = consts.tile([XA, HID], f32, tag="wlrx")
        wrlx_s = consts.tile([XA, HID], f32, tag="wrlx")
        wlr48_s = consts.tile([HC, HID], f32, tag="wlr48")
        wrl48_s = consts.tile([HC, HID], f32, tag="wrl48")
        for dst, src in ((wlrx_s, wlrx), (wrlx_s, wrlx),
                         (wlr48_s, wlr48), (wrl48_s, wrl48)):
            nc.sync.dma_start(out=dst[:, :], in_=src[:, :])

        # ---- chain buffers created early so their h0 DMAs beat the W load
        # into the FIFO sync queue; rows 0:16 = h states, 16:32 = 0 (junk
        # stripe), 32:48 = xproj (filled later) ----
        hx_lr = consts.tile([HC, (S + 1) * BL], f32, tag="hx_lr")
        hx_rl = consts.tile([HC, (S + 1) * BL], f32, tag="hx_rl")
        nc.vector.memset(hx_lr[:, :], 0.0)
        nc.vector.memset(hx_rl[:, :], 0.0)
        nc.sync.dma_start(out=hx_lr[0:HID, 0:BL], in_=h0c[:, :])
        nc.sync.dma_start(out=hx_rl[0:HID, 0:BL], in_=h0c[:, :])

        # ---- W_aug resident load (overlaps gather/xproj/RNN) ----
        waug_sb = consts.tile([128, WHALF], bf16, tag="waug")
        nc.sync.dma_start(out=waug_sb[:, :], in_=waug[:, :])

        # ---- gather embeddings, forward and step-reversed ----
        ers = []
        for k, nm in enumerate(("f0", "f1", "r0", "r1")):
            er = consts.tile([128, EMB], f32, tag=f"er_{nm}", name=f"er_{nm}")
            nc.gpsimd.indirect_dma_start(
                out=er[:, :], out_offset=None, in_=emb_tab[:, :],
                in_offset=bass.IndirectOffsetOnAxis(ap=idx[k][:, :1], axis=0))
            ers.append(er)

        ident = consts.tile([128, 128], f32, tag="ident")
        make_identity(nc, ident)

        # ---- x-major layouts with ones row: xa_f, xa_r [33, 256] ----
        xa_f = consts.tile([XA, R], f32, tag="xa_f")
        xa_r = consts.tile([XA, R], f32, tag="xa_r")
        for xa, e0, e1 in ((xa_f, ers[0], ers[1]), (xa_r, ers[2], ers[3])):
            nc.vector.memset(xa[EMB:XA, :], 1.0)
            for half, er in ((0, e0), (1, e1)):
                pt = p1.tile([EMB, 128], f32, tag="p1", name=f"pt{half}")
                nc.tensor.transpose(pt[:, :], er[:, :], ident[:, :])
                nc.vector.tensor_copy(out=xa[0:EMB, half * 128:(half + 1) * 128],
                                      in_=pt[:, :])

        # ---- xproj into the chain buffers' rows 32:48 ----
        for hx, w_s, xa in ((hx_lr, wlrx_s, xa_f), (hx_rl, wrlx_s, xa_r)):
            pp = p1.tile([HID, R], f32, tag="p1", name="xpj")
            nc.tensor.matmul(pp[:, :], lhsT=w_s[:, :], rhs=xa[:, :],
                             start=True, stop=True)
            nc.vector.tensor_copy(out=hx[2 * HID:HC, 0:R], in_=pp[:, :])

        # hRL pre-states rearranged into word order as they are produced
        h_rl = consts.tile([HID, R], f32, tag="h_rl")
        nc.vector.tensor_copy(out=h_rl[:, (S - 1) * BL:S * BL],
                              in_=hx_rl[0:HID, 0:BL])

        # ---- the two recurrences, interleaved (63 steps each) ----
        for t in range(S - 1):
            for hx, w48 in ((hx_lr, wlr48_s), (hx_rl, wrl48_s)):
                ps = p1.tile([HID, BL], f32, tag="p1",
                             name=f"r{'l' if hx is hx_lr else 'r'}{t}")
                nc.tensor.matmul(ps[:, :], lhsT=w48[:, :],
                                 rhs=hx[:, t * BL:(t + 1) * BL],
                                 start=True, stop=True)
                nc.scalar.activation(out=hx[0:HID, (t + 1) * BL:(t + 2) * BL],
                                     in_=ps[:, :], func=AF.Tanh)
            j = t + 1
            nc.vector.tensor_copy(
                out=h_rl[:, (S - 1 - j) * BL:(S - j) * BL],
                in_=hx_rl[0:HID, j * BL:(j + 1) * BL])

        # ---- haug = [hLR; hRL; 1] bf16, replicated to partitions 64:97 ----
        haug_f = consts.tile([KA, R], f32, tag="haug_f")
        nc.vector.tensor_copy(out=haug_f[0:HID, :], in_=hx_lr[0:HID, 0:R])
        # rows 16:32 aren't a legal compute-engine write target; DMA can
        nc.sync.dma_start(out=haug_f[HID:2 * HID, :], in_=h_rl[:, :])
        nc.vector.memset(haug_f[2 * HID:KA, :], 1.0)
        haug2 = consts.tile([97, R], bf16, tag="haug2")
        nc.vector.tensor_copy(out=haug2[0:KA, :], in_=haug_f[:, :])
        nc.vector.tensor_copy(out=haug2[64:64 + KA, :], in_=haug_f[:, :])

        # ---- per-row-group softmax state ----
        sums = [consts.tile([128, NT], f32, tag=f"sums{rc}", name=f"sums{rc}")
                for rc in range(2)]
        negl = [consts.tile([128, 1], f32, tag=f"negl{rc}", name=f"negl{rc}")
                for rc in range(2)]

        cache = {}

        def pass1_tile(rc, k, pfx):
            # tile k covers vocab chunks 4k..4k+3; even chunks live on W
            # partitions 0:33, odd on 64:97 (haug replicated to match)
            ps = p1.tile([128, TILEW], f32, tag="p1", name=f"{pfx}{k}")
            for q in range(4):
                base = 0 if q % 2 == 0 else 64
                j = 2 * k + q // 2
                nc.tensor.matmul(
                    ps[:, q * CHUNK:(q + 1) * CHUNK],
                    lhsT=haug2[base:base + KA, rc * 128:(rc + 1) * 128],
                    rhs=waug_sb[base:base + KA, j * CHUNK:(j + 1) * CHUNK],
                    start=True, stop=True)
            ca = cpool.tile([128, TILEW], bf16, tag="ca", name=f"ca{pfx}{k}")
            nc.vector.tensor_copy(out=ca[:, :], in_=ps[:, :])
            cache[(rc, k)] = ca
            # exp reads the bf16 cache, not psum: the psum slot frees right
            # after the copy, and exp trails asynchronously on ACT (only the
            # phase-end reduce waits on it).  Also normalizes the softmax
            # over the exact bf16 logits we store.
            ex = epool.tile([128, TILEW], bf16, tag="ex", name=f"ex{pfx}{k}")
            nc.scalar.activation(out=ex[:, :], in_=ca[:, :], func=AF.Exp,
                                 accum_out=sums[rc][:, k:k + 1])

        def finish_negl(rc):
            tot = consts.tile([128, 1], f32, tag=f"tot{rc}", name=f"tot{rc}")
            nc.vector.reduce_sum(out=tot[:, :], in_=sums[rc][:, 0:NT],
                                 axis=mybir.AxisListType.X)
            lnt = consts.tile([128, 1], f32, tag=f"lnt{rc}", name=f"lnt{rc}")
            nc.scalar.activation(out=lnt[:, :], in_=tot[:, :], func=AF.Ln)
            nc.vector.tensor_scalar_mul(out=negl[rc][:, :], in0=lnt[:, :],
                                        scalar1=-1.0)

        st_state = {}

        def pass2_tile(rc, k, on_act):
            ca = cache.pop((rc, k))
            g, kk = divmod(k, GRPT)
            if kk == 0:
                st_state[rc] = opool.tile([128, GRPT * TILEW], bf16, tag="st",
                                          name=f"st{rc}_{g}")
            st = st_state[rc]
            dst = st[:, kk * TILEW:(kk + 1) * TILEW]
            if on_act:
                nc.scalar.activation(out=dst, in_=ca[:, :], func=AF.Identity,
                                     bias=negl[rc][:, 0:1])
            else:
                nc.vector.tensor_scalar_add(out=dst, in0=ca[:, :],
                                            scalar1=negl[rc][:, 0:1])
            if kk == GRPT - 1 or k == NT - 1:
                c0g = g * GRPT * TILEW
                gw = (kk + 1) * TILEW
                cw = min(gw, V - c0g)
                nc.sync.dma_start(
                    out=out[rc * 128:(rc + 1) * 128, c0g:c0g + cw],
                    in_=st[:, 0:cw])

        # ---- phase A: pass1(rc0) ----
        for k in range(NT):
            pass1_tile(0, k, "a")
        finish_negl(0)

        # ---- phase B: pass1(rc1) || pass2(rc0) + store ----
        # pass2(rc0,k) reads cache slot k before pass1(rc1,k)'s copy
        # recycles it, so it must come first in (DVE) program order.
        for k in range(NT):
            pass2_tile(0, k, on_act=(k % 4 == 3))
            pass1_tile(1, k, "b")
        finish_negl(1)

        # ---- phase C: pass2(rc1) ----
        for k in range(NT):
            pass2_tile(1, k, on_act=(k % 3 == 2))

    nc.finalize()
    return nc


_NC = None


def get_nc():
    global _NC
    if _NC is None:
        _NC = build_nc()
    return _NC


def _make_waug(Who, bho):
    # [W_hLR; W_hRL; b_ho] packed two chunks deep: rows 0:33 hold even
    # vocab chunks, rows 64:97 odd chunks.  Pad columns carry bias -1e4 so
    # exp(logit) underflows to exactly 0.
    flat = np.zeros((KA, VPAD), dtype=np.float32)
    flat[0:2 * HID, :V] = Who.T
    flat[2 * HID, :V] = bho
    flat[2 * HID, V:] = -1e4
    packed = np.zeros((128, WHALF), dtype=np.float32)
    c = flat.reshape(KA, NCHUNKS, CHUNK)
    packed[0:KA] = c[:, 0::2, :].reshape(KA, WHALF)
    packed[64:64 + KA] = c[:, 1::2, :].reshape(KA, WHALF)
    return packed


def make_in_maps(**inputs):
    ib = np.asarray(inputs["input_batch"]).astype(np.int32)          # [S, B]
    emb = np.ascontiguousarray(np.asarray(inputs["embedding"], dtype=np.float32))
    Wlr = np.asarray(inputs["W_lr"], dtype=np.float32)               # [16, 48]
    Wrl = np.asarray(inputs["W_rl"], dtype=np.float32)
    blr = np.asarray(inputs["b_lr"], dtype=np.float32).reshape(1, HID)
    brl = np.asarray(inputs["b_rl"], dtype=np.float32).reshape(1, HID)
    Who = np.asarray(inputs["W_ho"], dtype=np.float32)               # [V, 32]
    bho = np.asarray(inputs["b_ho"], dtype=np.float32)               # [V]
    h0 = np.asarray(inputs["h0"], dtype=np.float32)                  # [1, 16]

    def w48(W):
        m = np.zeros((HC, HID), dtype=np.float32)
        m[0:HID] = W[:, EMB:].T
        m[2 * HID:HC] = np.eye(HID, dtype=np.float32)
        return m

    shared = dict(
        emb_tab=emb,
        waug=_make_waug(Who, bho).astype(mybir.dt.np(bf16)),
        wlrx=np.ascontiguousarray(np.concatenate([Wlr[:, :EMB].T, blr], axis=0)),
        wrlx=np.ascontiguousarray(np.concatenate([Wrl[:, :EMB].T, brl], axis=0)),
        wlr48=w48(Wlr),
        wrl48=w48(Wrl),
        h0c=np.ascontiguousarray(np.broadcast_to(h0.T, (HID, BL))),
    )
    in_maps = []
    for c in range(NCORES):
        cols = ib[:, c * BL:(c + 1) * BL]
        ind_f = np.ascontiguousarray(cols.reshape(R, 1))
        ind_r = np.ascontiguousarray(cols[::-1, :].reshape(R, 1))
        in_maps.append({**shared, "ind_f": ind_f, "ind_r": ind_r})
    return in_maps


def assemble(results):
    outs = [np.asarray(results[c]["out"], dtype=np.float32).reshape(S, BL, V)
            for c in range(NCORES)]
    return np.concatenate(outs, axis=1)


def kernel(**inputs):
    in_maps = make_in_maps(**inputs)
    res = run_bass_kernel_spmd(get_nc(), in_maps, list(range(NCORES)))
    return assemble(res.results)


if __name__ == "__main__":
    rng = np.random.default_rng(0)
    stdv = 1.0 / np.sqrt(HID)
    u = lambda *shp: rng.uniform(-stdv, stdv, shp).astype(np.float32)
    demo = dict(
        input_batch=rng.integers(0, V, (S, B)).astype(np.int32),
        embedding=u(V, EMB), W_lr=u(HID, EMB + HID), b_lr=u(HID),
        W_rl=u(HID, EMB + HID), b_rl=u(HID), W_ho=u(V, 2 * HID), b_ho=u(V),
        h0=u(1, HID),
    )
    out_arr = kernel(**demo)
    print(out_arr.shape, out_arr.dtype, float(out_arr[0, 0, :3].sum()))


# revision 19
# speedup vs baseline: 1.8765x; 1.0421x over previous
"""BiRNN language model on 8 Trainium2 NeuronCores (v3).

Model (see reference): emb lookup -> two tiny 16-wide RNNs (L->R and R->L,
collecting pre-update states) -> logits = [hLR|hRL] @ W_ho.T + b_ho over a
50257 vocab -> log_softmax.  Output [64, 32, 50257] (~412 MB) dominates:
memory-bound regime.

Sharding: data-parallel over batch (B=32 -> 4 columns/core).  Per core:
  1. W_aug = [W_ho.T; b_ho] is packed [97, VPAD/2] bf16 (even vocab chunks
     on partitions 0:33, odd on 64:97) so its one-time DMA uses 97 DMA
     partition lanes instead of 33, and stays resident in SBUF (~51 KB/part).
     haug is replicated to partitions 0:33 and 64:97 to match (matmul lhsT
     and rhs must share a partition base).
  2. Embeddings gathered twice (forward + step-reversed); each RNN direction
     runs as one K=48 matmul + one tanh per step: lhsT = [W_h.T; 0; I] reads
     [h; 0; xproj] so the xproj add costs nothing (no psum prefill).  The
     two chains interleave so engines stay busy.  Pre-update states are
     collected; RL states are copied into word order on the fly.
  3. pass1 per 128-row group rc: 4 chunk matmuls into a [128,2048] psum
     tile -> ACT exp with accum_out (per-row partial sums) -> DVE copies
     the raw logits to a bf16 SBUF cache ring (25 slots).  PE runs at its
     observed 1.2 GHz streaming rate; no recompute pass.
  4. pass2 per rc: out = cache + (-ln(sum)) as 4x-mode DVE tensor_scalar
     adds into bf16 staging tiles, DMA'd to HBM.  Phases pipeline:
     A = pass1(rc0); B = pass1(rc1) || pass2(rc0)+store; C = pass2(rc1).
  5. Output is bf16 in HBM (halves store traffic; ~2e-3 rel error, well
     inside tolerance); the host upcasts to f32.
No collectives; the host concatenates the 8 batch slices.
"""

import sys

sys.path.insert(0, "/opt/trn_rl_repo")

from contextlib import ExitStack

import numpy as np

import concourse.bass as bass
import concourse.bacc as bacc
import concourse.tile as tile
from concourse import mybir
from concourse.bass_utils import run_bass_kernel_spmd
from concourse.masks import make_identity

S, B, V, HID, EMB = 64, 32, 50257, 16, 32
NCORES = 8
BL = B // NCORES          # batch columns per core
R = S * BL                # logit rows per core (256 = 2 row-groups of 128)
XA = EMB + 1              # 33: [x; 1] contraction for the xproj precompute
KA = 2 * HID + 1          # 33: [hLR; hRL; 1] contraction for logits
CHUNK = 512               # vocab columns per matmul (one PSUM bank)
NCHUNKS = 100             # pad V to 100 chunks so the W pack is uniform
VPAD = NCHUNKS * CHUNK    # 51200
WHALF = VPAD // 2         # 25600 columns per W partition-group
TILEW = 4 * CHUNK         # psum tile width (4 chunks)
NT = NCHUNKS // 4         # 25 tiles per row-group
GRPT = 4                  # psum tiles per cache slot / store DMA (8192 cols)
NG = (NT + GRPT - 1) // GRPT
GRPW = GRPT * TILEW       # 8192
HC = 3 * HID              # 48: [h; 0; xp] recurrence contraction

f32 = mybir.dt.float32
bf16 = mybir.dt.bfloat16
i32 = mybir.dt.int32
AF = mybir.ActivationFunctionType


def build_nc():
    nc = bacc.Bacc()

    ind_f = nc.declare_dram_parameter("ind_f", [R, 1], i32, isOutput=False)
    ind_r = nc.declare_dram_parameter("ind_r", [R, 1], i32, isOutput=False)
    emb_tab = nc.declare_dram_parameter("emb_tab", [V, EMB], f32, isOutput=False)
    # [W_x.T; b] per direction for the xproj precompute
    wlrx = nc.declare_dram_parameter("wlrx", [XA, HID], f32, isOutput=False)
    wrlx = nc.declare_dram_parameter("wrlx", [XA, HID], f32, isOutput=False)
    # [W_h.T; 0; I] per direction for the recurrence
    wlr48 = nc.declare_dram_parameter("wlr48", [HC, HID], f32, isOutput=False)
    wrl48 = nc.declare_dram_parameter("wrl48", [HC, HID], f32, isOutput=False)
    h0c = nc.declare_dram_parameter("h0c", [HID, BL], f32, isOutput=False)
    # packed W_aug: rows 0:33 even chunks, rows 64:97 odd chunks; padded to
    # 128 partitions because few-partition long-line HBM reads serialize on
    # one SDMA engine (~26 GB/s) while 128-partition transfers go ~348 GB/s
    waug = nc.declare_dram_parameter("waug", [128, WHALF], bf16, isOutput=False)
    out = nc.declare_dram_parameter("out", [R, V], bf16, isOutput=True)

    with ExitStack() as ctx:
        tc = ctx.enter_context(tile.TileContext(nc))
        consts = ctx.enter_context(tc.tile_pool(name="consts", bufs=1))
        cpool = ctx.enter_context(tc.tile_pool(name="cpool", bufs=8))
        epool = ctx.enter_context(tc.tile_pool(name="epool", bufs=2))
        p1 = ctx.enter_context(tc.tile_pool(name="p1", bufs=2, space="PSUM"))

        # ---- small input DMAs first so nothing queues behind the W load ----
        idx = []
        for nm, src in (("if0", ind_f), ("if1", ind_f), ("ir0", ind_r),
                        ("ir1", ind_r)):
            t = consts.tile([128, 1], i32, tag=f"idx_{nm}", name=f"idx_{nm}")
            half = 1 if nm.endswith("1") else 0
            nc.sync.dma_start(out=t[:, :], in_=src[half * 128:(half + 1) * 128, :])
            idx.append(t)
        wlrx_s = consts.tile([XA, HID], f32, tag="wlrx")
        wrlx_s = consts.tile([XA, HID], f32, tag="wrlx")
        wlr48_s = consts.tile([HC, HID], f32, tag="wlr48")
        wrl48_s = consts.tile([HC, HID], f32, tag="wrl48")
        for dst, src in ((wlrx_s, wlrx), (wrlx_s, wrlx),
                         (wlr48_s, wlr48), (wrl48_s, wrl48)):
            nc.sync.dma_start(out=dst[:, :], in_=src[:, :])

        # ---- chain buffers created early so their h0 DMAs beat the W load
        # into the FIFO sync queue; rows 0:16 = h states, 16:32 = 0 (junk
        # stripe), 32:48 = xproj (filled later) ----
        hx_lr = consts.tile([HC, (S + 1) * BL], f32, tag="hx_lr")
        hx_rl = consts.tile([HC, (S + 1) * BL], f32, tag="hx_rl")
        nc.vector.memset(hx_lr[:, :], 0.0)
        nc.vector.memset(hx_rl[:, :], 0.0)
        nc.sync.dma_start(out=hx_lr[0:HID, 0:BL], in_=h0c[:, :])
        nc.sync.dma_start(out=hx_rl[0:HID, 0:BL], in_=h0c[:, :])

        # ---- W_aug resident load (overlaps gather/xproj/RNN) ----
        waug_sb = consts.tile([128, WHALF], bf16, tag="waug")
        nc.sync.dma_start(out=waug_sb[:, :], in_=waug[:, :])

        # ---- gather embeddings, forward and step-reversed ----
        ers = []
        for k, nm in enumerate(("f0", "f1", "r0", "r1")):
            er = consts.tile([128, EMB], f32, tag=f"er_{nm}", name=f"er_{nm}")
            nc.gpsimd.indirect_dma_start(
                out=er[:, :], out_offset=None, in_=emb_tab[:, :],
                in_offset=bass.IndirectOffsetOnAxis(ap=idx[k][:, :1], axis=0))
            ers.append(er)

        ident = consts.tile([128, 128], f32, tag="ident")
        make_identity(nc, ident)

        # ---- x-major layouts with ones row: xa_f, xa_r [33, 256] ----
        xa_f = consts.tile([XA, R], f32, tag="xa_f")
        xa_r = consts.tile([XA, R], f32, tag="xa_r")
        nc.vector.memset(xa_f[EMB:XA, :], 1.0)
        nc.vector.memset(xa_r[EMB:XA, :], 1.0)

        def xa_half(xa, er, half):
            pt = p1.tile([EMB, 128], f32, tag="p1", name=f"pt{half}")
            nc.tensor.transpose(pt[:, :], er[:, :], ident[:, :])
            nc.vector.tensor_copy(out=xa[0:EMB, half * 128:(half + 1) * 128],
                                  in_=pt[:, :])

        # ---- xproj into the chain buffers' rows 32:48, one half at a time
        # so the LR chain can start before the reverse gathers finish ----
        def xproj_half(hx, w_s, xa, half):
            lo, hi = half * 128, (half + 1) * 128
            pp = p1.tile([HID, 128], f32, tag="p1", name=f"xpj{half}")
            nc.tensor.matmul(pp[:, :], lhsT=w_s[:, :], rhs=xa[:, lo:hi],
                             start=True, stop=True)
            nc.vector.tensor_copy(out=hx[2 * HID:HC, lo:hi], in_=pp[:, :])

        for half in range(2):
            xa_half(xa_f, ers[half], half)
            xproj_half(hx_lr, wlrx_s, xa_f, half)
        for half in range(2):
            xa_half(xa_r, ers[2 + half], half)
            xproj_half(hx_rl, wrlx_s, xa_r, half)

        # hRL pre-states rearranged into word order as they are produced
        h_rl = consts.tile([HID, R], f32, tag="h_rl")
        nc.vector.tensor_copy(out=h_rl[:, (S - 1) * BL:S * BL],
                              in_=hx_rl[0:HID, 0:BL])

        # ---- the two recurrences, interleaved (63 steps each) ----
        for t in range(S - 1):
            for hx, w48 in ((hx_lr, wlr48_s), (hx_rl, wrl48_s)):
                ps = p1.tile([HID, BL], f32, tag="p1",
                             name=f"r{'l' if hx is hx_lr else 'r'}{t}")
                nc.tensor.matmul(ps[:, :], lhsT=w48[:, :],
                                 rhs=hx[:, t * BL:(t + 1) * BL],
                                 start=True, stop=True)
                nc.scalar.activation(out=hx[0:HID, (t + 1) * BL:(t + 2) * BL],
                                     in_=ps[:, :], func=AF.Tanh)
            j = t + 1
            nc.vector.tensor_copy(
                out=h_rl[:, (S - 1 - j) * BL:(S - j) * BL],
                in_=hx_rl[0:HID, j * BL:(j + 1) * BL])

        # ---- haug = [hLR; hRL; 1] bf16, replicated to partitions 64:97 ----
        haug_f = consts.tile([KA, R], f32, tag="haug_f")
        nc.vector.tensor_copy(out=haug_f[0:HID, :], in_=hx_lr[0:HID, 0:R])
        # rows 16:32 aren't a legal compute-engine write target; DMA can
        nc.sync.dma_start(out=haug_f[HID:2 * HID, :], in_=h_rl[:, :])
        nc.vector.memset(haug_f[2 * HID:KA, :], 1.0)
        haug2 = consts.tile([97, R], bf16, tag="haug2")
        nc.vector.tensor_copy(out=haug2[0:KA, :], in_=haug_f[:, :])
        nc.vector.tensor_copy(out=haug2[64:64 + KA, :], in_=haug_f[:, :])

        # ---- per-row-group softmax state ----
        sums = [consts.tile([128, NT], f32, tag=f"sums{rc}", name=f"sums{rc}")
                for rc in range(2)]
        negl = [consts.tile([128, 1], f32, tag=f"negl{rc}", name=f"negl{rc}")
                for rc in range(2)]

        cache = {}

        def pass1_tile(rc, k, pfx, cast_act=False):
            # tile k covers vocab chunks 4k..4k+3; even chunks live on W
            # partitions 0:33, odd on 64:97 (haug replicated to match)
            ps = p1.tile([128, TILEW], f32, tag="p1", name=f"{pfx}{k}")
            for q in range(4):
                base = 0 if q % 2 == 0 else 64
                j = 2 * k + q // 2
                nc.tensor.matmul(
                    ps[:, q * CHUNK:(q + 1) * CHUNK],
                    lhsT=haug2[base:base + KA, rc * 128:(rc + 1) * 128],
                    rhs=waug_sb[base:base + KA, j * CHUNK:(j + 1) * CHUNK],
                    start=True, stop=True)
            g, kk = divmod(k, GRPT)
            if kk == 0:
                cache[(rc, g)] = cpool.tile([128, GRPW], bf16, tag="ca",
                                            name=f"ca{pfx}{g}")
            dst = cache[(rc, g)][:, kk * TILEW:(kk + 1) * TILEW]
            # copy psum -> bf16 cache; psum slot frees right after (exp reads
            # the cache, so it trails asynchronously on ACT — only the
            # phase-end reduce waits on it).  Also normalizes the softmax
            # over the exact bf16 logits we store.
            if cast_act:
                nc.scalar.activation(out=dst, in_=ps[:, :], func=AF.Identity)
            else:
                nc.vector.tensor_copy(out=dst, in_=ps[:, :])
            ex = epool.tile([128, TILEW], bf16, tag="ex", name=f"ex{pfx}{k}")
            nc.scalar.activation(out=ex[:, :], in_=dst, func=AF.Exp,
                                 accum_out=sums[rc][:, k:k + 1])

        def finish_negl(rc):
            tot = consts.tile([128, 1], f32, tag=f"tot{rc}", name=f"tot{rc}")
            nc.vector.reduce_sum(out=tot[:, :], in_=sums[rc][:, 0:NT],
                                 axis=mybir.AxisListType.X)
            lnt = consts.tile([128, 1], f32, tag=f"lnt{rc}", name=f"lnt{rc}")
            nc.scalar.activation(out=lnt[:, :], in_=tot[:, :], func=AF.Ln)
            nc.vector.tensor_scalar_mul(out=negl[rc][:, :], in0=lnt[:, :],
                                        scalar1=-1.0)

        def pass2_group(rc, g):
            # one in-place 4x-mode add of -ln(sum) over the whole cache
            # slot, then DMA it straight to HBM (no separate staging)
            ca = cache.pop((rc, g))
            gw = min(GRPW, (NT - g * GRPT) * TILEW)
            nc.vector.tensor_scalar_add(out=ca[:, 0:gw], in0=ca[:, 0:gw],
                                        scalar1=negl[rc][:, 0:1])
            c0g = g * GRPW
            cw = min(gw, V - c0g)
            nc.sync.dma_start(
                out=out[rc * 128:(rc + 1) * 128, c0g:c0g + cw],
                in_=ca[:, 0:cw])

        # ---- phase A: pass1(rc0) ----
        for k in range(NT):
            pass1_tile(0, k, "a")
        finish_negl(0)

        # ---- phase B: pass1(rc1) || pass2(rc0) + store ----
        # pass2(rc0,g) consumes cache slot g before pass1(rc1)'s copies
        # recycle it, so it comes first in (DVE) program order; every 6th
        # cache copy runs on ACT to balance the add load DVE picks up.
        for k in range(NT):
            g, kk = divmod(k, GRPT)
            if kk == 0:
                pass2_group(0, g)
            pass1_tile(1, k, "b", cast_act=(k % 6 == 5))
        finish_negl(1)

        # ---- phase C: pass2(rc1) ----
        for g in range(NG):
            pass2_group(1, g)

    nc.finalize()
    return nc


_NC = None


def get_nc():
    global _NC
    if _NC is None:
        _NC = build_nc()
    return _NC


def _make_waug(Who, bho):
    # [W_hLR; W_hRL; b_ho] packed two chunks deep: rows 0:33 hold even
    # vocab chunks, rows 64:97 odd chunks.  Pad columns carry bias -1e4 so
    # exp(logit) underflows to exactly 0.
    flat = np.zeros((KA, VPAD), dtype=np.float32)
    flat[0:2 * HID, :V] = Who.T
    flat[2 * HID, :V] = bho
    flat[2 * HID, V:] = -1e4
    packed = np.zeros((128, WHALF), dtype=np.float32)
    c = flat.reshape(KA, NCHUNKS, CHUNK)
    packed[0:KA] = c[:, 0::2, :].reshape(KA, WHALF)
    packed[64:64 + KA] = c[:, 1::2, :].reshape(KA, WHALF)
    return packed


def make_in_maps(**inputs):
    ib = np.asarray(inputs["input_batch"]).astype(np.int32)          # [S, B]
    emb = np.ascontiguousarray(np.asarray(inputs["embedding"], dtype=np.float32))
    Wlr = np.asarray(inputs["W_lr"], dtype=np.float32)               # [16, 48]
    Wrl = np.asarray(inputs["W_rl"], dtype=np.float32)
    blr = np.asarray(inputs["b_lr"], dtype=np.float32).reshape(1, HID)
    brl = np.asarray(inputs["b_rl"], dtype=np.float32).reshape(1, HID)
    Who = np.asarray(inputs["W_ho"], dtype=np.float32)               # [V, 32]
    bho = np.asarray(inputs["b_ho"], dtype=np.float32)               # [V]
    h0 = np.asarray(inputs["h0"], dtype=np.float32)                  # [1, 16]

    def w48(W):
        m = np.zeros((HC, HID), dtype=np.float32)
        m[0:HID] = W[:, EMB:].T
        m[2 * HID:HC] = np.eye(HID, dtype=np.float32)
        return m

    shared = dict(
        emb_tab=emb,
        waug=_make_waug(Who, bho).astype(mybir.dt.np(bf16)),
        wlrx=np.ascontiguousarray(np.concatenate([Wlr[:, :EMB].T, blr], axis=0)),
        wrlx=np.ascontiguousarray(np.concatenate([Wrl[:, :EMB].T, brl], axis=0)),
        wlr48=w48(Wlr),
        wrl48=w48(Wrl),
        h0c=np.ascontiguousarray(np.broadcast_to(h0.T, (HID, BL))),
    )
    in_maps = []
    for c in range(NCORES):
        cols = ib[:, c * BL:(c + 1) * BL]
        ind_f = np.ascontiguousarray(cols.reshape(R, 1))
        ind_r = np.ascontiguousarray(cols[::-1, :].reshape(R, 1))
        in_maps.append({**shared, "ind_f": ind_f, "ind_r": ind_r})
    return in_maps


def assemble(results):
    outs = [np.asarray(results[c]["out"], dtype=np.float32).reshape(S, BL, V)
            for c in range(NCORES)]
    return np.concatenate(outs, axis=1)


def kernel(**inputs):
    in_maps = make_in_maps(**inputs)
    res = run_bass_kernel_spmd(get_nc(), in_maps, list(range(NCORES)))
    return assemble(res.results)


if __name__ == "__main__":
    rng = np.random.default_rng(0)
    stdv = 1.0 / np.sqrt(HID)
    u = lambda *shp: rng.uniform(-stdv, stdv, shp).astype(np.float32)
    demo = dict(
        input_batch=rng.integers(0, V, (S, B)).astype(np.int32),
        embedding=u(V, EMB), W_lr=u(HID, EMB + HID), b_lr=u(HID),
        W_rl=u(HID, EMB + HID), b_rl=u(HID), W_ho=u(V, 2 * HID), b_ho=u(V),
        h0=u(1, HID),
    )
    out_arr = kernel(**demo)
    print(out_arr.shape, out_arr.dtype, float(out_arr[0, 0, :3].sum()))
